# revision 46
# baseline (speedup 1.0000x reference)
"""Conv2d 3x3 VALID kernel for Trainium2, batch-sharded across 8 NeuronCores.

Problem: input [32,128,64,64] f32, weights [256,128,3,3] f32 ->
output [32,256,62,62] f32 (stride 1, no padding).

Strategy (per core, 4 images): 1-D Winograd F(2,3) along H + direct taps
along W, matmuls in bf16 (rel-err budget 2e-2, measured ~6e-3).

  For output row pair (2y', 2y'+1), with d_k = in[2y'+k] (rows) and per-kw
  column taps g0..g2 handled by shifted rhs views:
    V0 = d0 - d2, V1 = d1 + d2, V2 = d2 - d1, V3 = d1 - d3   (4 bf16 planes)
    U0 = g0, U1 = (g0+g1+g2)/2, U2 = (g0-g1+g2)/2, U3 = g2   (weights, bf16)
    m_e[y', x] = sum_kw sum_ci U_e,kw[ci,co] * V_e[ci, y', x+kw]  (PE, PSUM)
    out[2y']   = m0 + m1 + m2
    out[2y'+1] = m1 - m2 - m3
  12 matmul passes per 16 output rows instead of the direct method's 18:
  PE row count drops 1.5x (276,768 -> 184,512 rows; ~77us at 1 col/cycle).

Engine budget per image (~20us of matmuls):
  ACT:    fp32->bf16 input converts (2) + one 4-plane PSUM->SBUF bf16 copy
          per chunk (the only PSUM reader, so the 4-bank PSUM slot recycles
          in ~1.8us < 2.5us of the next chunk's matmuls; pool bufs=2).
  DVE:    V planes (bf16 2x_1P) + output combines per chunk
          (t_p=c1+c2, t_m=c1-c2, o_even=c0+t_p, o_odd=t_m-c3).
  GPSIMD: nothing. Its SBUF port is shared with the Vector engine and any
          streaming GPSIMD op slows concurrent DVE ops ~8x (measured).
"""

import numpy as np

import concourse.bass as bass
import concourse.mybir as mybir
import concourse.tile as tile
from concourse import bacc
from concourse.alu_op_type import AluOpType
from concourse.bass_utils import run_bass_kernel_spmd
from concourse.masks import make_identity

F32 = mybir.dt.float32
BF16 = mybir.dt.bfloat16

B, CIN, H, W = 32, 128, 64, 64
COUT, KH, KW = 256, 3, 3
OH, OW = H - KH + 1, W - KW + 1  # 62, 62
N_CORES = 8
BL = B // N_CORES  # 4 images per core

IMG = H * W  # 4096
W_FREE = CIN * KH * KW  # 1152
N_TAPS = KH * KW  # 9
YT = OH // 2  # 31 y' tiles
VPLANE = YT * W  # 1984 elements per V plane
CHUNKS = [(0, 8), (8, 8), (16, 8), (24, 7)]  # (y'0, n_tiles)


def _weight_taps(nc, tc, psum_pool, w_raw, w_t, ident, h, copy_eng):
    """PE-transpose half h's taps to [ci, co]; copy out on copy_eng.

    w_t layout: [ci, tap*256 + h*128 + co] (fp32, tap = kh*3+kw)
    """
    w_v = w_raw[:, h * W_FREE : (h + 1) * W_FREE].rearrange(
        "p (ci t) -> p t ci", t=N_TAPS
    )
    # All 9 taps transpose into ONE 4-bank tile (4 per bank; start=True
    # clears only the bank's has_written bits, not data), so the PE runs
    # them back-to-back with no ring pacing on the trailing copies, which
    # batch per bank (taps are 128-contiguous in PSUM, stride-256 in w_t).
    ps = psum_pool.tile([128, 4 * 512], F32, tag="m", name=f"tps_{h}")
    for t in range(N_TAPS):
        off = (t // 4) * 512 + (t % 4) * 128
        nc.tensor.transpose(ps[:, off : off + 128], w_v[:, t, :], ident)
    w_t_v = w_t.rearrange("p (t x) -> p t x", x=COUT)
    for bank in range(3):
        n = min(4, N_TAPS - 4 * bank)
        src = ps[:, bank * 512 : bank * 512 + n * 128].rearrange(
            "p (t x) -> p t x", x=128
        )
        dst = w_t_v[:, 4 * bank : 4 * bank + n, h * 128 : h * 128 + 128]
        if copy_eng == "act":
            nc.scalar.copy(dst, src)
        else:
            nc.vector.tensor_copy(dst, src)
    return ps


def _weight_u03(nc, tc, w_t, u_l, h):
    """u0/u3 are bf16 copies of the kh=0/kh=2 taps, batched across kw."""
    w_t_v = w_t.rearrange("p (t x) -> p t x", x=COUT)
    u_l_v = u_l.rearrange("p (t x) -> p t x", x=COUT)
    cs = slice(h * 128, h * 128 + 128)
    nc.vector.tensor_copy(u_l_v[:, 0:3, cs], w_t_v[:, 0:3, cs])
    nc.vector.tensor_copy(u_l_v[:, 9:12, cs], w_t_v[:, 6:9, cs])


def _weight_combos(nc, tc, wtmp_pool, w_t, u_l, h):
    """Build the u1/u2 Winograd combinations, batched across kw (DVE)."""
    w_t_v = w_t.rearrange("p (t x) -> p t x", x=COUT)
    u_l_v = u_l.rearrange("p (t x) -> p t x", x=COUT)
    cs = slice(h * 128, h * 128 + 128)
    s3 = wtmp_pool.tile([128, 3 * 128], F32, tag="wtmp", name=f"s3_{h}")
    q3 = wtmp_pool.tile([128, 3 * 128], F32, tag="wtmp", name=f"q3_{h}")
    s3_v = s3.rearrange("p (t x) -> p t x", x=128)
    q3_v = q3.rearrange("p (t x) -> p t x", x=128)
    nc.vector.tensor_add(s3_v, w_t_v[:, 0:3, cs], w_t_v[:, 6:9, cs])
    nc.vector.tensor_scalar_mul(q3_v, w_t_v[:, 3:6, cs], 0.5)
    nc.vector.scalar_tensor_tensor(
        u_l_v[:, 3:6, cs], s3_v, 0.5, q3_v, AluOpType.mult, AluOpType.add
    )
    nc.vector.scalar_tensor_tensor(
        u_l_v[:, 6:9, cs], s3_v, 0.5, q3_v, AluOpType.mult, AluOpType.subtract
    )


def _conv_body(nc, tc, out_d, x_d, w_d):
    x_r = x_d.rearrange("b c h w -> b c (h w)")  # [BL, 128, 4096]

    with (
        tc.tile_pool(name="const", bufs=1) as cpool,
        tc.tile_pool(name="psum", bufs=2, space=bass.MemorySpace.PSUM) as psum_pool,
        tc.tile_pool(name="outp", bufs=3) as out_pool,
        tc.tile_pool(name="tmp", bufs=3) as tmp_pool,
    ):
        stage = cpool.tile([128, 2 * IMG], F32)  # rolling fp32 DMA landing
        in_bf = cpool.tile([128, 2 * IMG], BF16)  # rolling bf16 image
        v_all = cpool.tile([128, 2 * 4 * VPLANE], BF16)  # rolling V planes
        w_raw = cpool.tile([128, 2 * W_FREE], F32)
        w_t = cpool.tile([128, N_TAPS * COUT], F32)
        u_l = cpool.tile([128, 12 * COUT], BF16)
        ident = cpool.tile([128, 128], F32)

        make_identity(nc, ident)
        w_r = w_d.rearrange("co ci kh kw -> co (ci kh kw)")  # [256, 1152]

        def v_planes(b, spans):
            sl = (b % 2) * IMG
            dv = in_bf[:, sl : sl + IMG].rearrange("p (r x) -> p r x", x=W)
            for y0, yn in spans:
                r0 = 2 * y0
                rn = 2 * yn

                def rows(k):
                    return dv[:, r0 + k : r0 + k + rn - 1 : 2, :]

                for e, (ra, rb, op) in enumerate(
                    ((0, 2, "sub"), (1, 2, "add"), (2, 1, "sub"), (1, 3, "sub"))
                ):
                    vout = v_all[
                        :,
                        ((b % 2) * 4 + e) * VPLANE
                        + y0 * W : ((b % 2) * 4 + e) * VPLANE
                        + (y0 + yn) * W,
                    ].rearrange("p (y x) -> p y x", x=W)
                    fn = nc.vector.tensor_add if op == "add" else nc.vector.tensor_sub
                    fn(vout, rows(ra), rows(rb))

        def pf_dma(b):
            sl = (b % 2) * IMG
            for c0, c1 in ((0, IMG // 2), (IMG // 2, IMG)):
                nc.sync.dma_start(
                    out=stage[:, sl + c0 : sl + c1], in_=x_r[b][:, c0:c1]
                )

        def pf_convert(b, piece):
            sl = (b % 2) * IMG
            c0, c1 = ((0, IMG // 2), (IMG // 2, IMG))[piece]
            nc.scalar.copy(
                in_bf[:, sl + c0 : sl + c1], stage[:, sl + c0 : sl + c1]
            )

        # Startup: half-0 weights land first (smaller DMA), image 0 next,
        # half-1 weights last. DVE order is arranged so ops gating the
        # first matmuls run earliest: half-0 tap copies + u0/u3, then V0's
        # first two chunk-quarters, half-0 combos, V0's rest. Half-1 tap
        # copies run on ACT (it is idle after the image-0 converts).
        nc.sync.dma_start(
            out=w_raw[:, :W_FREE], in_=w_r[0:128, :]
        )
        nc.sync.dma_start(
            out=w_raw[:, W_FREE : 2 * W_FREE], in_=w_r[128:256, :]
        )
        sl0 = 0
        for c0, c1 in ((0, IMG // 2), (IMG // 2, IMG)):
            nc.sync.dma_start(out=stage[:, sl0 + c0 : sl0 + c1], in_=x_r[0][:, c0:c1])
            nc.scalar.copy(
                in_bf[:, sl0 + c0 : sl0 + c1], stage[:, sl0 + c0 : sl0 + c1]
            )
        _weight_taps(nc, tc, psum_pool, w_raw, w_t, ident, 0, "dve")
        _weight_u03(nc, tc, w_t, u_l, 0)
        v_planes(0, ((0, 8), (8, 8)))
        _weight_combos(nc, tc, tmp_pool, w_t, u_l, 0)
        ps1 = _weight_taps(nc, tc, psum_pool, w_raw, w_t, ident, 1, "act")
        _weight_u03(nc, tc, w_t, u_l, 1)
        _weight_combos(nc, tc, tmp_pool, w_t, u_l, 1)
        v_planes(0, ((16, 8), (24, 7)))
        # Warm-up matmuls into the transpose tile's unused 4th bank: the PE
        # would otherwise idle ~3.6us waiting on the DVE prep chain, which
        # exceeds the HAM activity window and re-throttles the clock to
        # 1.2GHz for the first conv matmuls (transposes don't count as
        # PE-busy for HAM). Inputs are ready early; output is never read.
        for wi in range(6):
            nc.tensor.matmul(
                ps1[:, 1536 : 1536 + 496],
                u_l[:, :128],
                in_bf[:, :496],
                start=True,
                stop=True,
            )

        for b in range(BL):
            # Image 0 interleaves the Cout halves: (h0,c0),(h0,c1),(h1,c0),
            # (h1,c1) reuse the V quarters built at startup, giving the DVE
            # ~5us to finish q3/q4 before (h0,c2) needs them. Other images
            # (whose V planes are fully prefetched) run h0 then h1.
            if b == 0:
                seq = (
                    [(0, c) for c in CHUNKS[:2]]
                    + [(1, c) for c in CHUNKS[:2]]
                    + [(0, c) for c in CHUNKS[2:]]
                    + [(1, c) for c in CHUNKS[2:]]
                )
            else:
                seq = [(h, c) for h in range(2) for c in CHUNKS]
            for ei, (h, (y0, ny)) in enumerate(seq):
                # Prefetch of image b+1 is spread across the second half of
                # the block so its converts/V-ops don't head-of-line-block
                # the ACT/DVE FIFOs ahead of the PSUM-draining cm copies.
                pre = b + 1 < BL and ei >= 4
                if ei == 4 and b + 1 < BL:
                    pf_dma(b + 1)
                if True:
                    size = ny * OW
                    m = psum_pool.tile([128, 4 * 512], F32, tag="m", name="m")
                    for e in range(4):
                        vv = v_all[
                            :,
                            ((b % 2) * 4 + e)
                            * VPLANE : ((b % 2) * 4 + e + 1)
                            * VPLANE,
                        ].rearrange("p (y x) -> p y x", x=W)
                        me_v = m[:, e * 512 : e * 512 + size].rearrange(
                            "p (y x) -> p y x", x=OW
                        )
                        for kw in range(KW):
                            lhsT = u_l[:, (e * 3 + kw) * COUT + h * 128 :][
                                :, :128
                            ]
                            nc.tensor.matmul(
                                me_v,
                                lhsT,
                                vv[:, y0 : y0 + ny, kw : kw + OW],
                                start=(kw == 0),
                                stop=(kw == KW - 1),
                            )
                    # Sole PSUM reader: batched 4-plane bf16 copy, so the
                    # 4-bank slot recycles in ~1.8us < the next chunk's MMs.
                    cm = tmp_pool.tile([128, 4 * 496], BF16, tag="cm", name="cm")
                    nc.scalar.copy(
                        cm.rearrange("p (e k) -> p e k", k=496)[:, :, :size],
                        m.rearrange("p (e k) -> p e k", k=512)[:, :, :size],
                    )
                    t12 = tmp_pool.tile(
                        [128, 2 * 496], BF16, tag="t12", name="t12"
                    )
                    c0 = cm[:, 0:size]
                    c1 = cm[:, 496 : 496 + size]
                    c2 = cm[:, 992 : 992 + size]
                    c3 = cm[:, 1488 : 1488 + size]
                    t_p = t12[:, 0:size]
                    t_m = t12[:, 496 : 496 + size]
                    nc.vector.tensor_add(t_p, c1, c2)
                    nc.vector.tensor_sub(t_m, c1, c2)
                    ot = out_pool.tile([128, 16 * OW], F32, name="ot")
                    ot_v = ot[:, : 2 * ny * OW].rearrange("p (y x) -> p y x", x=OW)

                    def v3(ap):
                        return ap.rearrange("p (y x) -> p y x", x=OW)

                    nc.vector.tensor_add(
                        ot_v[:, 0 : 2 * ny : 2, :], v3(c0), v3(t_p)
                    )
                    nc.vector.tensor_sub(
                        ot_v[:, 1 : 2 * ny : 2, :], v3(t_m), v3(c3)
                    )
                    nc.sync.dma_start(
                        out=out_d[
                            b,
                            h * 128 : (h + 1) * 128,
                            2 * y0 : 2 * y0 + 2 * ny,
                            :,
                        ],
                        in_=ot_v,
                    )
                    if pre:
                        if ei < 6:
                            pf_convert(b + 1, ei - 4)
                        elif ei == 6:
                            v_planes(b + 1, ((0, 16),))
                        else:
                            v_planes(b + 1, ((16, 15),))


def build_module():
    nc = bacc.Bacc(
        "TRN2", target_bir_lowering=False, debug=False, num_devices=N_CORES
    )
    x_d = nc.dram_tensor(
        "input_image", [BL, CIN, H, W], F32, kind="ExternalInput"
    ).ap()
    w_d = nc.dram_tensor("weights", [COUT, CIN, KH, KW], F32, kind="ExternalInput").ap()
    out_d = nc.dram_tensor("out", [BL, COUT, OH, OW], F32, kind="ExternalOutput").ap()
    with tile.TileContext(nc) as tc:
        _conv_body(nc, tc, out_d, x_d, w_d)
    nc.compile()
    return nc


_NC_CACHE = {}


def _get_module():
    if "m" not in _NC_CACHE:
        _NC_CACHE["m"] = build_module()
    return _NC_CACHE["m"]


def kernel(input_image: np.ndarray, weights: np.ndarray) -> np.ndarray:
    input_image = np.ascontiguousarray(input_image, dtype=np.float32)
    weights = np.ascontiguousarray(weights, dtype=np.float32)
    nc = _get_module()
    in_maps = [
        {
            "input_image": input_image[i * BL : (i + 1) * BL],
            "weights": weights,
        }
        for i in range(N_CORES)
    ]
    res = run_bass_kernel_spmd(nc, in_maps, list(range(N_CORES))).results
    return np.concatenate([r["out"] for r in res], axis=0)


# revision 49
# speedup vs baseline: 1.0248x; 1.0248x over previous
"""Conv2d 3x3 VALID kernel for Trainium2, batch-sharded across 8 NeuronCores.

Problem: input [32,128,64,64] f32, weights [256,128,3,3] f32 ->
output [32,256,62,62] f32 (stride 1, no padding).

Strategy (per core, 4 images): 1-D Winograd F(2,3) along H + direct taps
along W, matmuls in bf16 (rel-err budget 2e-2, measured ~6e-3).

  For output row pair (2y', 2y'+1), with d_k = in[2y'+k] (rows) and per-kw
  column taps g0..g2 handled by shifted rhs views:
    V0 = d0 - d2, V1 = d1 + d2, V2 = d2 - d1, V3 = d1 - d3   (4 bf16 planes)
    U0 = g0, U1 = (g0+g1+g2)/2, U2 = (g0-g1+g2)/2, U3 = g2   (weights, bf16)
    m_e[y', x] = sum_kw sum_ci U_e,kw[ci,co] * V_e[ci, y', x+kw]  (PE, PSUM)
    out[2y']   = m0 + m1 + m2
    out[2y'+1] = m1 - m2 - m3
  12 matmul passes per 16 output rows instead of the direct method's 18:
  PE row count drops 1.5x (276,768 -> 184,512 rows; ~77us at 1 col/cycle).

Engine budget per image (~20us of matmuls):
  ACT:    fp32->bf16 input converts (2) + one 4-plane PSUM->SBUF bf16 copy
          per chunk (the only PSUM reader, so the 4-bank PSUM slot recycles
          in ~1.8us < 2.5us of the next chunk's matmuls; pool bufs=2).
  DVE:    V planes (bf16 2x_1P) + output combines per chunk
          (t_p=c1+c2, t_m=c1-c2, o_even=c0+t_p, o_odd=t_m-c3).
  GPSIMD: nothing. Its SBUF port is shared with the Vector engine and any
          streaming GPSIMD op slows concurrent DVE ops ~8x (measured).
"""

import numpy as np

import concourse.bass as bass
import concourse.mybir as mybir
import concourse.tile as tile
from concourse import bacc
from concourse.alu_op_type import AluOpType
from concourse.bass_utils import run_bass_kernel_spmd
from concourse.masks import make_identity

F32 = mybir.dt.float32
BF16 = mybir.dt.bfloat16

B, CIN, H, W = 32, 128, 64, 64
COUT, KH, KW = 256, 3, 3
OH, OW = H - KH + 1, W - KW + 1  # 62, 62
N_CORES = 8
BL = B // N_CORES  # 4 images per core

IMG = H * W  # 4096
W_FREE = CIN * KH * KW  # 1152
N_TAPS = KH * KW  # 9
YT = OH // 2  # 31 y' tiles
VPLANE = YT * W  # 1984 elements per V plane
CHUNKS = [(0, 8), (8, 8), (16, 8), (24, 7)]  # (y'0, n_tiles)


def _weight_taps(nc, tc, psum_pool, w_raw, w_t, ident, h, copy_eng):
    """PE-transpose half h's taps to [ci, co]; copy out on copy_eng.

    w_t layout: [ci, tap*256 + h*128 + co] (fp32, tap = kh*3+kw)
    """
    w_v = w_raw[:, h * W_FREE : (h + 1) * W_FREE].rearrange(
        "p (ci t) -> p t ci", t=N_TAPS
    )
    # All 9 taps transpose into ONE 4-bank tile (4 per bank; start=True
    # clears only the bank's has_written bits, not data), so the PE runs
    # them back-to-back with no ring pacing on the trailing copies, which
    # batch per bank (taps are 128-contiguous in PSUM, stride-256 in w_t).
    ps = psum_pool.tile([128, 4 * 512], F32, tag="m", name=f"tps_{h}")
    for t in range(N_TAPS):
        off = (t // 4) * 512 + (t % 4) * 128
        nc.tensor.transpose(ps[:, off : off + 128], w_v[:, t, :], ident)
    w_t_v = w_t.rearrange("p (t x) -> p t x", x=COUT)
    for bank in range(3):
        n = min(4, N_TAPS - 4 * bank)
        src = ps[:, bank * 512 : bank * 512 + n * 128].rearrange(
            "p (t x) -> p t x", x=128
        )
        dst = w_t_v[:, 4 * bank : 4 * bank + n, h * 128 : h * 128 + 128]
        if copy_eng == "act":
            nc.scalar.copy(dst, src)
        else:
            nc.vector.tensor_copy(dst, src)


def _weight_u03(nc, tc, w_t, u_l, h):
    """u0/u3 are bf16 copies of the kh=0/kh=2 taps, batched across kw."""
    w_t_v = w_t.rearrange("p (t x) -> p t x", x=COUT)
    u_l_v = u_l.rearrange("p (t x) -> p t x", x=COUT)
    cs = slice(h * 128, h * 128 + 128)
    nc.vector.tensor_copy(u_l_v[:, 0:3, cs], w_t_v[:, 0:3, cs])
    nc.vector.tensor_copy(u_l_v[:, 9:12, cs], w_t_v[:, 6:9, cs])


def _weight_combos(nc, tc, wtmp_pool, w_t, u_l, h):
    """Build the u1/u2 Winograd combinations, batched across kw (DVE)."""
    w_t_v = w_t.rearrange("p (t x) -> p t x", x=COUT)
    u_l_v = u_l.rearrange("p (t x) -> p t x", x=COUT)
    cs = slice(h * 128, h * 128 + 128)
    s3 = wtmp_pool.tile([128, 3 * 128], F32, tag="wtmp", name=f"s3_{h}")
    q3 = wtmp_pool.tile([128, 3 * 128], F32, tag="wtmp", name=f"q3_{h}")
    s3_v = s3.rearrange("p (t x) -> p t x", x=128)
    q3_v = q3.rearrange("p (t x) -> p t x", x=128)
    nc.vector.tensor_add(s3_v, w_t_v[:, 0:3, cs], w_t_v[:, 6:9, cs])
    nc.vector.tensor_scalar_mul(q3_v, w_t_v[:, 3:6, cs], 0.5)
    nc.vector.scalar_tensor_tensor(
        u_l_v[:, 3:6, cs], s3_v, 0.5, q3_v, AluOpType.mult, AluOpType.add
    )
    nc.vector.scalar_tensor_tensor(
        u_l_v[:, 6:9, cs], s3_v, 0.5, q3_v, AluOpType.mult, AluOpType.subtract
    )


def _conv_body(nc, tc, out_d, x_d, w_d):
    x_r = x_d.rearrange("b c h w -> b c (h w)")  # [BL, 128, 4096]

    with (
        tc.tile_pool(name="const", bufs=1) as cpool,
        tc.tile_pool(name="psum", bufs=2, space=bass.MemorySpace.PSUM) as psum_pool,
        tc.tile_pool(name="outp", bufs=3) as out_pool,
        tc.tile_pool(name="tmp", bufs=3) as tmp_pool,
    ):
        stage = cpool.tile([128, 2 * IMG], F32)  # rolling fp32 DMA landing
        in_bf = cpool.tile([128, 2 * IMG], BF16)  # rolling bf16 image
        v_all = cpool.tile([128, 2 * 4 * VPLANE], BF16)  # rolling V planes
        w_raw = cpool.tile([128, 2 * W_FREE], F32)
        w_t = cpool.tile([128, N_TAPS * COUT], F32)
        u_l = cpool.tile([128, 12 * COUT], BF16)
        ident = cpool.tile([128, 128], F32)

        make_identity(nc, ident)
        w_r = w_d.rearrange("co ci kh kw -> co (ci kh kw)")  # [256, 1152]

        def v_planes(b, spans):
            sl = (b % 2) * IMG
            dv = in_bf[:, sl : sl + IMG].rearrange("p (r x) -> p r x", x=W)
            for y0, yn in spans:
                r0 = 2 * y0
                rn = 2 * yn

                def rows(k):
                    return dv[:, r0 + k : r0 + k + rn - 1 : 2, :]

                for e, (ra, rb, op) in enumerate(
                    ((0, 2, "sub"), (1, 2, "add"), (2, 1, "sub"), (1, 3, "sub"))
                ):
                    vout = v_all[
                        :,
                        ((b % 2) * 4 + e) * VPLANE
                        + y0 * W : ((b % 2) * 4 + e) * VPLANE
                        + (y0 + yn) * W,
                    ].rearrange("p (y x) -> p y x", x=W)
                    fn = nc.vector.tensor_add if op == "add" else nc.vector.tensor_sub
                    fn(vout, rows(ra), rows(rb))

        def pf_dma(b):
            sl = (b % 2) * IMG
            for c0, c1 in ((0, IMG // 2), (IMG // 2, IMG)):
                nc.sync.dma_start(
                    out=stage[:, sl + c0 : sl + c1], in_=x_r[b][:, c0:c1]
                )

        def pf_convert(b, piece):
            sl = (b % 2) * IMG
            c0, c1 = ((0, IMG // 2), (IMG // 2, IMG))[piece]
            nc.scalar.copy(
                in_bf[:, sl + c0 : sl + c1], stage[:, sl + c0 : sl + c1]
            )

        # Startup: half-0 weights land first (smaller DMA), image 0 next,
        # half-1 weights last. DVE order is arranged so ops gating the
        # first matmuls run earliest: half-0 tap copies + u0/u3, then V0's
        # first two chunk-quarters, half-0 combos, V0's rest. Half-1 tap
        # copies run on ACT (it is idle after the image-0 converts).
        nc.sync.dma_start(
            out=w_raw[:, :W_FREE], in_=w_r[0:128, :]
        )
        nc.sync.dma_start(
            out=w_raw[:, W_FREE : 2 * W_FREE], in_=w_r[128:256, :]
        )
        sl0 = 0
        for c0, c1 in ((0, IMG // 2), (IMG // 2, IMG)):
            nc.sync.dma_start(out=stage[:, sl0 + c0 : sl0 + c1], in_=x_r[0][:, c0:c1])
            nc.scalar.copy(
                in_bf[:, sl0 + c0 : sl0 + c1], stage[:, sl0 + c0 : sl0 + c1]
            )
        _weight_taps(nc, tc, psum_pool, w_raw, w_t, ident, 0, "dve")
        _weight_u03(nc, tc, w_t, u_l, 0)
        v_planes(0, ((0, 8), (8, 8)))
        _weight_combos(nc, tc, tmp_pool, w_t, u_l, 0)
        _weight_taps(nc, tc, psum_pool, w_raw, w_t, ident, 1, "act")
        _weight_u03(nc, tc, w_t, u_l, 1)
        _weight_combos(nc, tc, tmp_pool, w_t, u_l, 1)
        v_planes(0, ((16, 8), (24, 7)))

        for b in range(BL):
            # Image 0 interleaves the Cout halves: (h0,c0),(h0,c1),(h1,c0),
            # (h1,c1) reuse the V quarters built at startup, giving the DVE
            # ~5us to finish q3/q4 before (h0,c2) needs them. Other images
            # (whose V planes are fully prefetched) run h0 then h1.
            if b == 0:
                seq = (
                    [(0, c) for c in CHUNKS[:2]]
                    + [(1, c) for c in CHUNKS[:2]]
                    + [(0, c) for c in CHUNKS[2:]]
                    + [(1, c) for c in CHUNKS[2:]]
                )
            else:
                seq = [(h, c) for h in range(2) for c in CHUNKS]
            for ei, (h, (y0, ny)) in enumerate(seq):
                # Prefetch of image b+1 is spread across the second half of
                # the block so its converts/V-ops don't head-of-line-block
                # the ACT/DVE FIFOs ahead of the PSUM-draining cm copies.
                pre = b + 1 < BL and ei >= 4
                if ei == 4 and b + 1 < BL:
                    pf_dma(b + 1)
                if True:
                    size = ny * OW
                    m = psum_pool.tile([128, 4 * 512], F32, tag="m", name="m")
                    for e in range(4):
                        vv = v_all[
                            :,
                            ((b % 2) * 4 + e)
                            * VPLANE : ((b % 2) * 4 + e + 1)
                            * VPLANE,
                        ].rearrange("p (y x) -> p y x", x=W)
                        me_v = m[:, e * 512 : e * 512 + size].rearrange(
                            "p (y x) -> p y x", x=OW
                        )
                        for kw in range(KW):
                            lhsT = u_l[:, (e * 3 + kw) * COUT + h * 128 :][
                                :, :128
                            ]
                            nc.tensor.matmul(
                                me_v,
                                lhsT,
                                vv[:, y0 : y0 + ny, kw : kw + OW],
                                start=(kw == 0),
                                stop=(kw == KW - 1),
                            )
                    # Sole PSUM reader: batched 4-plane bf16 copy, so the
                    # 4-bank slot recycles in ~1.8us < the next chunk's MMs.
                    cm = tmp_pool.tile([128, 4 * 496], BF16, tag="cm", name="cm")
                    nc.scalar.copy(
                        cm.rearrange("p (e k) -> p e k", k=496)[:, :, :size],
                        m.rearrange("p (e k) -> p e k", k=512)[:, :, :size],
                    )
                    t12 = tmp_pool.tile(
                        [128, 2 * 496], BF16, tag="t12", name="t12"
                    )
                    c0 = cm[:, 0:size]
                    c1 = cm[:, 496 : 496 + size]
                    c2 = cm[:, 992 : 992 + size]
                    c3 = cm[:, 1488 : 1488 + size]
                    t_p = t12[:, 0:size]
                    t_m = t12[:, 496 : 496 + size]
                    nc.vector.tensor_add(t_p, c1, c2)
                    nc.vector.tensor_sub(t_m, c1, c2)
                    ot = out_pool.tile([128, 16 * OW], F32, name="ot")
                    ot_v = ot[:, : 2 * ny * OW].rearrange("p (y x) -> p y x", x=OW)

                    def v3(ap):
                        return ap.rearrange("p (y x) -> p y x", x=OW)

                    nc.vector.tensor_add(
                        ot_v[:, 0 : 2 * ny : 2, :], v3(c0), v3(t_p)
                    )
                    nc.vector.tensor_sub(
                        ot_v[:, 1 : 2 * ny : 2, :], v3(t_m), v3(c3)
                    )
                    nc.sync.dma_start(
                        out=out_d[
                            b,
                            h * 128 : (h + 1) * 128,
                            2 * y0 : 2 * y0 + 2 * ny,
                            :,
                        ],
                        in_=ot_v,
                    )
                    if pre:
                        if ei < 6:
                            pf_convert(b + 1, ei - 4)
                        elif ei == 6:
                            v_planes(b + 1, ((0, 16),))
                        else:
                            v_planes(b + 1, ((16, 15),))


def build_module():
    nc = bacc.Bacc(
        "TRN2", target_bir_lowering=False, debug=False, num_devices=N_CORES
    )
    x_d = nc.dram_tensor(
        "input_image", [BL, CIN, H, W], F32, kind="ExternalInput"
    ).ap()
    w_d = nc.dram_tensor("weights", [COUT, CIN, KH, KW], F32, kind="ExternalInput").ap()
    out_d = nc.dram_tensor("out", [BL, COUT, OH, OW], F32, kind="ExternalOutput").ap()
    with tile.TileContext(nc) as tc:
        _conv_body(nc, tc, out_d, x_d, w_d)
    nc.compile()
    return nc


_NC_CACHE = {}


def _get_module():
    if "m" not in _NC_CACHE:
        _NC_CACHE["m"] = build_module()
    return _NC_CACHE["m"]


def kernel(input_image: np.ndarray, weights: np.ndarray) -> np.ndarray:
    input_image = np.ascontiguousarray(input_image, dtype=np.float32)
    weights = np.ascontiguousarray(weights, dtype=np.float32)
    nc = _get_module()
    in_maps = [
        {
            "input_image": input_image[i * BL : (i + 1) * BL],
            "weights": weights,
        }
        for i in range(N_CORES)
    ]
    res = run_bass_kernel_spmd(nc, in_maps, list(range(N_CORES))).results
    return np.concatenate([r["out"] for r in res], axis=0)
